# revision 21
# baseline (speedup 1.0000x reference)
"""Trainium2 Bass kernel for nn_CustomConv2D (degenerate conv: only the last
input channel contributes; 3x3 VALID conv -> 64 out channels + bias).

Strategy:
  - Host: slice x_padded[:, -1] (the only channel the reference uses), build
    the 9-row im2col matrix per batch (cheap: 29 MB total), shard batch dim
    across 8 cores (8 batches per core).
  - Device (per core): one [128, 3136] moving tile per batch PAIR holds the
    pair's im2col matrix [18, 12544] split into 4 pixel segments placed at
    partition offsets 0/32/64/96 (one contiguous DMA, full port spread).
    Stationary weight [128, 128] is block-diagonal over the pair (cols 0-63
    batch A channels, 64-127 batch B) and replicated at the 4 partition
    offsets. Each segment runs 7 fp32 matmuls (N=448) at tile_position
    (32s, 0) -> PSUM [128, 448]; bias is fused into the PSUM->SBUF
    evacuation (alternating VectorE tensor_scalar_add / ScalarE activation
    Identity), and each segment's [128, 3136] staging tile streams out as a
    1.6 MiB DMA.
"""

import sys

if "/opt/trn_rl_repo" not in sys.path:
    sys.path.insert(0, "/opt/trn_rl_repo")

import numpy as np

B, CIN, COUT, KS = 64, 64, 64, 3
H, W, HP, WP = 112, 112, 114, 114
NPIX = H * W          # 12544
IMG = HP * WP         # 12996
NCORES = 8
BL = B // NCORES      # 8 local batches per core
PAIRS = BL // 2       # 4
KDIM = 2 * KS * KS    # 18
NSEG = 4              # pixel segments per pair (partition offsets 0/32/64/96)
SEGW = NPIX // NSEG   # 3136
NT = 448              # pixels per matmul; 7 * 448 == 3136, fits one PSUM bank
TPS = SEGW // NT      # 7 matmul tiles per segment

_CACHE = {}


def _build_bass():
    import concourse.bass as bass
    import concourse.bacc as bacc
    import concourse.mybir as mybir
    from concourse.tile import TileContext

    f32 = mybir.dt.float32
    f32r = mybir.dt.float32r
    # Bacc (not plain Bass): its compile() runs move_matmul_waits_to_ldweights
    # + generate_event_semaphores, without which walrus rejects any sync wait
    # on a Matmult ("Too many sync wait commands").
    nc = bacc.Bacc("TRN2", target_bir_lowering=False, debug=False)
    mv = nc.declare_dram_parameter("mv", [PAIRS, 128, SEGW], f32r,
                                   isOutput=False)
    w2 = nc.declare_dram_parameter("w2", [128, 128], f32r, isOutput=False)
    b2 = nc.declare_dram_parameter("b2", [128, 1], f32, isOutput=False)
    out = nc.declare_dram_parameter("out", [BL * COUT, NPIX], f32,
                                    isOutput=True)

    with TileContext(nc) as tc:
        with (
            tc.tile_pool(name="consts", bufs=1) as consts,
            tc.tile_pool(name="movp", bufs=3) as movp,
            tc.tile_pool(name="stagep", bufs=10) as stagep,
            tc.tile_pool(name="psump", bufs=8, space="PSUM") as psump,
        ):
            w2_t = consts.tile([128, 128], f32r)
            nc.gpsimd.dma_start(out=w2_t[:], in_=w2[:])
            b2_t = consts.tile([128, 1], f32)
            nc.gpsimd.dma_start(out=b2_t[:], in_=b2[:])



            tidx = 0
            for pair in range(PAIRS):
                # 32-row groups arrive fully (rows 18-31 zero-filled from
                # host; their weight rows are zero too). Per-seg DMAs let
                # each segment's matmuls start as soon as its rows land;
                # pair 0 seg 0 rides SWDGE (no TENSOR_LOAD dependency, so it
                # issues right after the preamble barrier).
                mov = movp.tile([128, SEGW + 32], f32r, tag="mov")
                for s4 in range(NSEG):
                    eng = nc.gpsimd if (pair == 0 and s4 == 0) else nc.scalar
                    eng.dma_start(
                        out=mov[32 * s4:32 * (s4 + 1), 0:SEGW],
                        in_=mv[pair, 32 * s4:32 * (s4 + 1), :])

                # t-major emission: consecutive matmuls hit different
                # 32-row groups, so up to 4 run concurrently in the PE array.
                stages = [stagep.tile([128, SEGW], f32, tag="stage",
                                      name=f"stage_{pair}_{s}")
                          for s in range(NSEG)]
                for t in range(TPS):
                    n0 = t * NT
                    for seg in range(NSEG):
                        p0 = 32 * seg
                        ps = psump.tile([128, NT], f32, tag="ps")
                        nc.tensor.matmul(ps[:, :],
                                         w2_t[p0:p0 + KDIM, :],
                                         mov[p0:p0 + KDIM, n0:n0 + NT],
                                         start=True, stop=True,
                                         tile_position=(p0, 0))
                        # PSUM -> SBUF with fused bias add; alternate engines.
                        if tidx % 2 == 0:
                            nc.vector.tensor_scalar_add(
                                stages[seg][:, n0:n0 + NT], ps[:, :],
                                b2_t[:, :])
                        else:
                            nc.scalar.activation(
                                stages[seg][:, n0:n0 + NT], ps[:, :],
                                mybir.ActivationFunctionType.Identity,
                                bias=b2_t[:, :])
                        tidx += 1
                    if t == 3:
                        # first 4 columns-of-448 of every stage are done:
                        # start draining while t=4..6 compute
                        for seg in range(NSEG):
                            nc.sync.dma_start(
                                out=out[pair * 128:(pair + 1) * 128,
                                        seg * SEGW:seg * SEGW + 4 * NT],
                                in_=stages[seg][:, 0:4 * NT])
                for seg in range(NSEG):
                    nc.sync.dma_start(
                        out=out[pair * 128:(pair + 1) * 128,
                                seg * SEGW + 4 * NT:(seg + 1) * SEGW],
                        in_=stages[seg][:, 4 * NT:SEGW])
    nc.compile()
    return nc


def _get_nc():
    if "nc" not in _CACHE:
        _CACHE["nc"] = _build_bass()
    return _CACHE["nc"]


def _prep_inputs(x_padded, weight, bias):
    x = np.asarray(x_padded, dtype=np.float32)
    wt = np.asarray(weight, dtype=np.float32)
    bs = np.asarray(bias, dtype=np.float32)

    xs3 = x[:, -1, :, :]                              # [64, 114, 114]
    win = np.lib.stride_tricks.sliding_window_view(xs3, (KS, KS), axis=(1, 2))
    # [64, 112, 112, 3, 3] -> [64, 9, 12544] with row k = (i, j) shift
    mov_all = win.transpose(0, 3, 4, 1, 2).reshape(B, KS * KS, NPIX)
    # [cores, pairs, 18, NSEG, SEGW] -> [cores, pairs, NSEG, 32, SEGW]
    mov_r = mov_all.reshape(NCORES, PAIRS, KDIM, NSEG, SEGW).transpose(0, 1, 3, 2, 4)
    mov_h = np.zeros((NCORES, PAIRS, NSEG, 32, SEGW), np.float32)
    mov_h[:, :, :, :KDIM, :] = mov_r
    mov_h = mov_h.reshape(NCORES, PAIRS, 128, SEGW)

    wl = np.ascontiguousarray(wt[:, -1, :, :]).reshape(COUT, KS * KS)
    w2 = np.zeros((128, 128), np.float32)
    for s in range(NSEG):
        w2[32 * s: 32 * s + 9, 0:64] = wl.T
        w2[32 * s + 9: 32 * s + 18, 64:128] = wl.T
    b2 = np.tile(bs, 2).reshape(128, 1).astype(np.float32)
    return mov_h, w2, b2


def kernel(x_padded, weight, bias, in_height=112, in_width=112, **_unused):
    from concourse.bass_utils import run_bass_kernel_spmd

    mov_h, w2, b2 = _prep_inputs(x_padded, weight, bias)
    nc = _get_nc()
    in_maps = [
        {"mv": mov_h[c], "w2": w2, "b2": b2}
        for c in range(NCORES)
    ]
    res = run_bass_kernel_spmd(nc, in_maps, core_ids=list(range(NCORES)))
    outs = [
        np.asarray(res.results[c]["out"]).reshape(BL, COUT, H, W)
        for c in range(NCORES)
    ]
    return np.concatenate(outs, axis=0)
